# revision 6
# baseline (speedup 1.0000x reference)
"""Trainium2 (trn2) Bass kernel for the DDSP noise-synthesis module.

Problem (hardcoded; no external files read):
  x           [32, 64, 16384] f32
  noise_w     [129, 64], noise_b [129] (zeros), noise_factor scalar
  white_noise [32, 16384]
  out[b, 0, t] = mean_c x[b, c, t] + noise_factor * noise_bank(spec_b, white_b)[t]
  spec_b = avgpool_128(clip(noise_w @ x_b + noise_b, 0, 1))        # [129, 128]
  noise_bank: per-frame rFFT(256, ortho) filtering of white noise + 50% overlap-add.
  (The reference's amp/freq oscillator branch is computed-but-discarded dead code.)

Strategy: pure data parallel, 4 batches per core (2 pairs).  Per pair the
channel dims of two batches are stacked to 128 partitions and the 1x1 conv
runs as three stationary-weight matmul passes (coeffs 0-63, 64-127 both
batches block-diagonal; Nyquist+mean rows via 32-col masked stationaries
accumulated into one PSUM tile).  clip+pool: ScalarE relu(+bias)+bf16 cast,
VectorE min(1) + segmented reduce.  rFFT/irFFT are 128/256-point DFT matmuls
with the overlap-add folded into PSUM accumulation (shifted filter columns),
and noise_factor/pool/ortho scales folded into the iDFT constants.
"""

import numpy as np

B, CH, T = 32, 64, 16384
NCORES = 8
BLOC = B // NCORES          # 4 batches per core
PAIRS = BLOC // 2           # 2
K = 129                     # rfft coeffs
S = 128                     # frames
WIN = 256
HOP = 128
SEG = 128                   # pool window
XCH = 4096                  # x stream chunk (free elems)
NQ = T // XCH               # 4
CCH = 512                   # conv free chunk = 1 PSUM bank of f32
NT = T // CCH               # 32

_CACHE: dict = {}


def _dft_consts(noise_factor: float):
    n = np.arange(WIN)[:, None].astype(np.float64)
    k = np.arange(K)[None, :].astype(np.float64)
    ang = 2.0 * np.pi * n * k / WIN
    Ar = (np.cos(ang) / 16.0).astype(np.float32)           # [256, 129]
    Ai = (-np.sin(ang) / 16.0).astype(np.float32)
    wk = np.full((K,), 2.0)
    wk[0] = 1.0
    wk[K - 1] = 1.0
    scale = noise_factor / (16.0 * 128.0)                  # ortho irfft + pool mean
    ang2 = 2.0 * np.pi * np.arange(K)[:, None] * np.arange(WIN)[None, :] / WIN
    Cr = (wk[:, None] * np.cos(ang2) * scale).astype(np.float32)   # [129, 256]
    Ci = (-wk[:, None] * np.sin(ang2) * scale).astype(np.float32)
    return Ar, Ai, Cr, Ci


def _build(reps: int = 1):
    from contextlib import ExitStack

    import concourse.bacc as bacc
    import concourse.bass as bass
    import concourse.tile as tile
    from concourse import mybir

    f32 = mybir.dt.float32
    bf16 = mybir.dt.bfloat16
    AF = mybir.ActivationFunctionType
    ALU = mybir.AluOpType
    AX = mybir.AxisListType

    nc = bacc.Bacc("TRN2", target_bir_lowering=False, debug=False,
                   num_devices=NCORES)

    xd = nc.dram_tensor("x", [PAIRS, 128, T], f32, kind="ExternalInput")
    wnd = nc.dram_tensor("wn", [BLOC, T + HOP], f32, kind="ExternalInput")
    w1d = nc.dram_tensor("w1", [128, 128], f32, kind="ExternalInput")
    w2d = nc.dram_tensor("w2", [128, 128], f32, kind="ExternalInput")
    wzd = nc.dram_tensor("wz", [128, 256], f32, kind="ExternalInput")
    b1d = nc.dram_tensor("b1", [128, 1], f32, kind="ExternalInput")
    b2d = nc.dram_tensor("b2", [128, 1], f32, kind="ExternalInput")
    arAd = nc.dram_tensor("arA", [128, K], f32, kind="ExternalInput")
    arBd = nc.dram_tensor("arB", [128, K], f32, kind="ExternalInput")
    aiAd = nc.dram_tensor("aiA", [128, K], f32, kind="ExternalInput")
    aiBd = nc.dram_tensor("aiB", [128, K], f32, kind="ExternalInput")
    crmd = nc.dram_tensor("crm", [128, WIN], f32, kind="ExternalInput")
    crnd = nc.dram_tensor("crn", [1, WIN], f32, kind="ExternalInput")
    cimd = nc.dram_tensor("cim", [128, WIN], f32, kind="ExternalInput")
    identd = nc.dram_tensor("ident", [128, 128], f32, kind="ExternalInput")
    yd = nc.dram_tensor("y", [BLOC, T], f32, kind="ExternalOutput")

    with tile.TileContext(nc) as tc, ExitStack() as ctx:
        consts = ctx.enter_context(tc.tile_pool(name="consts", bufs=1))
        xpool = ctx.enter_context(tc.tile_pool(name="xp", bufs=2 * NQ))
        ring = ctx.enter_context(tc.tile_pool(name="ring", bufs=6))
        specp = ctx.enter_context(tc.tile_pool(name="spec", bufs=4))
        qp = ctx.enter_context(tc.tile_pool(name="qp", bufs=2))
        sb = ctx.enter_context(tc.tile_pool(name="sb", bufs=2))
        outp = ctx.enter_context(tc.tile_pool(name="outp", bufs=2))
        pmm = ctx.enter_context(tc.tile_pool(name="pmm", bufs=3, space="PSUM"))
        pp3 = ctx.enter_context(tc.tile_pool(name="pp3", bufs=2, space="PSUM"))
        pnz = ctx.enter_context(tc.tile_pool(name="pnz", bufs=3, space="PSUM"))

        def cload(dram, shape, tag):
            t = consts.tile(shape, f32, tag=tag)
            nc.sync.dma_start(out=t, in_=dram[:, :])
            return t

        w1t = cload(w1d, [128, 128], "w1")
        w2t = cload(w2d, [128, 128], "w2")
        wzt = cload(wzd, [128, 256], "wz")
        b1t = cload(b1d, [128, 1], "b1")
        b2t = cload(b2d, [128, 1], "b2")
        arAt = cload(arAd, [128, K], "arA")
        arBt = cload(arBd, [128, K], "arB")
        aiAt = cload(aiAd, [128, K], "aiA")
        aiBt = cload(aiBd, [128, K], "aiB")
        crmt = cload(crmd, [128, WIN], "crm")
        crnt = cload(crnd, [1, WIN], "crn")
        cimt = cload(cimd, [128, WIN], "cim")
        identt = cload(identd, [128, 128], "ident")

        for _rep in range(reps):
            for pair in range(PAIRS):
                # ---- stream x for this batch pair ----
                xq = []
                for q in range(NQ):
                    t = xpool.tile([128, XCH], f32)
                    nc.sync.dma_start(
                        out=t, in_=xd[pair, :, q * XCH:(q + 1) * XCH])
                    xq.append(t)

                # ---- conv passes 1/2: coeffs 0-63 / 64-127 (both batches) ----
                spec = []
                for wt, bt in ((w1t, b1t), (w2t, b2t)):
                    sp = specp.tile([128, S], f32)
                    for ti in range(NT):
                        q, off = divmod(ti * CCH, XCH)
                        ps = pmm.tile([128, CCH], f32)
                        nc.tensor.matmul(ps, wt, xq[q][:, off:off + CCH],
                                         start=True, stop=True)
                        rb = ring.tile([128, CCH], bf16)
                        nc.scalar.activation(rb, ps, AF.Relu, bias=bt, scale=1.0)
                        nc.vector.tensor_scalar_min(rb, rb, 1.0)
                        nc.vector.tensor_reduce(
                            sp[:, 4 * ti:4 * ti + 4],
                            rb.rearrange("p (a b) -> p a b", b=SEG),
                            axis=AX.X, op=ALU.add)
                    spec.append(sp)

                # ---- pass 3: [nyqA, meanA, nyqB, meanB] x 32 chunks ----
                p3 = pp3.tile([128, CCH], f32)
                for ti in range(NT):
                    g, tau = divmod(ti, 8)
                    q, off = divmod(ti * CCH, XCH)
                    nc.tensor.matmul(
                        p3[32 * g:32 * g + 32, :],
                        wzt[:, 32 * tau:32 * tau + 32],
                        xq[q][:, off:off + CCH],
                        start=(tau == 0), stop=(tau == 7),
                        tile_position=(0, 32 * g))
                qt = qp.tile([128, CCH], f32)
                nc.vector.tensor_copy(qt, p3)

                # ---- per-batch noise bank + output ----
                for i in range(2):
                    b = 2 * pair + i

                    # compact nyq/mean rows of this batch: [32 chunks, 512]
                    qn = sb.tile([32, CCH], f32, tag="qn")
                    nc.sync.dma_start(out=qn, in_=qt[2 * i::4, :])
                    qm = sb.tile([32, CCH], f32, tag="qm")
                    nc.sync.dma_start(out=qm, in_=qt[2 * i + 1::4, :])

                    # spec for this batch: [128 k, 128 s]
                    spb = sb.tile([128, S], f32, tag="spb")
                    nc.sync.dma_start(out=spb[0:64, :],
                                      in_=spec[0][64 * i:64 * i + 64, :])
                    nc.sync.dma_start(out=spb[64:128, :],
                                      in_=spec[1][64 * i:64 * i + 64, :])

                    # nyquist spec row: clip + pool on qn, then lay out [1, 128]
                    qnc = sb.tile([32, CCH], f32, tag="qnc")
                    nc.vector.tensor_scalar(qnc, qn, 0.0, 1.0,
                                            op0=ALU.max, op1=ALU.min)
                    sgrid = sb.tile([32, 4], f32, tag="sgrid")
                    nc.vector.tensor_reduce(
                        sgrid, qnc.rearrange("p (a b) -> p a b", b=SEG),
                        axis=AX.X, op=ALU.add)
                    nyqrow = sb.tile([1, S], f32, tag="nyqrow")
                    nc.sync.dma_start(
                        out=bass.AP(tensor=nyqrow.tensor, offset=nyqrow.offset,
                                    ap=[[1, 1], [4, 32], [1, 4]]),
                        in_=bass.AP(tensor=sgrid.tensor, offset=sgrid.offset,
                                    ap=[[1, 1]] + [list(d) for d in sgrid.ap]))

                    # frames of white noise (overlapped windows) + transpose
                    wn_b = wnd[b, :]
                    frames = sb.tile([S, WIN], f32, tag="frames")
                    nc.sync.dma_start(
                        out=frames,
                        in_=bass.AP(tensor=wn_b.tensor, offset=wn_b.offset,
                                    ap=[[HOP, S], [1, WIN]]))
                    fT = []
                    for h in range(2):
                        tr = pnz.tile([128, 128], f32, tag="ps_nz")
                        nc.tensor.transpose(
                            tr, frames[:, 128 * h:128 * h + 128], identt)
                        ft = sb.tile([128, 128], f32, tag=f"ft{h}")
                        nc.vector.tensor_copy(ft, tr)
                        fT.append(ft)

                    # rfft: nf[k, s] (main 0..127 and nyquist row)
                    nfr = pnz.tile([128, 128], f32, tag="ps_nz")
                    nc.tensor.matmul(nfr, arAt[:, 0:128], fT[0],
                                     start=True, stop=False)
                    nc.tensor.matmul(nfr, arBt[:, 0:128], fT[1],
                                     start=False, stop=True)
                    nfi = pnz.tile([128, 128], f32, tag="ps_nz")
                    nc.tensor.matmul(nfi, aiAt[:, 0:128], fT[0],
                                     start=True, stop=False)
                    nc.tensor.matmul(nfi, aiBt[:, 0:128], fT[1],
                                     start=False, stop=True)
                    nfn = pnz.tile([1, 128], f32, tag="ps_nz")
                    nc.tensor.matmul(nfn, arAt[:, 128:129], fT[0],
                                     start=True, stop=False)
                    nc.tensor.matmul(nfn, arBt[:, 128:129], fT[1],
                                     start=False, stop=True)

                    # filt = nf * spec ; shifted copies for the OLA tail
                    fr = sb.tile([128, S], f32, tag="fr")
                    nc.vector.tensor_mul(fr, nfr, spb)
                    fi = sb.tile([128, S], f32, tag="fi")
                    nc.vector.tensor_mul(fi, nfi, spb)
                    fn = sb.tile([1, S], f32, tag="fn")
                    nc.vector.tensor_mul(fn, nfn, nyqrow)
                    frs = sb.tile([128, S], f32, tag="frs")
                    nc.gpsimd.memset(frs[:, 0:1], 0.0)
                    nc.gpsimd.tensor_copy(frs[:, 1:S], fr[:, 0:S - 1])
                    fis = sb.tile([128, S], f32, tag="fis")
                    nc.gpsimd.memset(fis[:, 0:1], 0.0)
                    nc.gpsimd.tensor_copy(fis[:, 1:S], fi[:, 0:S - 1])
                    fns = sb.tile([1, S], f32, tag="fns")
                    nc.gpsimd.memset(fns[:, 0:1], 0.0)
                    nc.gpsimd.tensor_copy(fns[:, 1:S], fn[:, 0:S - 1])

                    # transposed irfft + OLA, noise_factor prescaled: ola_T[j, s]
                    olaT = pnz.tile([128, 128], f32, tag="ps_nz")
                    nc.tensor.matmul(olaT, crmt[:, 0:128], fr,
                                     start=True, stop=False)
                    nc.tensor.matmul(olaT, crmt[:, 128:256], frs,
                                     start=False, stop=False)
                    nc.tensor.matmul(olaT, cimt[:, 0:128], fi,
                                     start=False, stop=False)
                    nc.tensor.matmul(olaT, cimt[:, 128:256], fis,
                                     start=False, stop=False)
                    nc.tensor.matmul(olaT, crnt[:, 0:128], fn,
                                     start=False, stop=False)
                    nc.tensor.matmul(olaT, crnt[:, 128:256], fns,
                                     start=False, stop=True)

                    # mean rows -> mean_T[j, (u, t)] via PE transposes
                    meanT = pnz.tile([128, 128], f32, tag="ps_nz")
                    for u in range(4):
                        nc.tensor.transpose(
                            meanT[:, 32 * u:32 * u + 32],
                            qm[:, 128 * u:128 * u + 128],
                            identt[0:32, 0:32])
                    meanTs = sb.tile([128, 128], f32, tag="meanTs")
                    nc.vector.tensor_copy(meanTs, meanT)

                    # F[j, s] = olaT + meanT  (s = 4t + u; meanT free is (u, t))
                    F = sb.tile([128, 128], f32, tag="F")
                    nc.vector.tensor_add(
                        bass.AP(tensor=F.tensor, offset=F.offset,
                                ap=[list(F.ap[0]), [4, 32], [1, 4]]),
                        bass.AP(tensor=olaT.tensor, offset=olaT.offset,
                                ap=[list(olaT.ap[0]), [4, 32], [1, 4]]),
                        bass.AP(tensor=meanTs.tensor, offset=meanTs.offset,
                                ap=[list(meanTs.ap[0]), [1, 32], [32, 4]]))

                    # transpose back to [s, j] and store
                    Ft = pnz.tile([128, 128], f32, tag="ps_nz")
                    nc.tensor.transpose(Ft, F, identt)
                    osb = outp.tile([128, 128], f32)
                    nc.vector.tensor_copy(osb, Ft)
                    yb = yd[b, :]
                    nc.sync.dma_start(
                        out=bass.AP(tensor=yb.tensor, offset=yb.offset,
                                    ap=[[128, 128], [1, 128]]),
                        in_=osb)

    nc.compile()
    return nc


def _host_prep(x, noise_w, noise_b, noise_factor, white_noise):
    W = np.ascontiguousarray(noise_w, np.float32)          # [129, 64]
    nb = np.asarray(noise_b, np.float32)
    nf = float(np.asarray(noise_factor, np.float32))
    Ar, Ai, Cr, Ci = _dft_consts(nf)

    w1 = np.zeros((128, 128), np.float32)
    w1[0:64, 0:64] = W[0:64].T
    w1[64:128, 64:128] = W[0:64].T
    w2 = np.zeros((128, 128), np.float32)
    w2[0:64, 0:64] = W[64:128].T
    w2[64:128, 64:128] = W[64:128].T
    wz = np.zeros((128, 256), np.float32)
    for tau in range(8):
        blk = wz[:, 32 * tau:32 * tau + 32]
        blk[0:64, 4 * tau + 0] = W[128]
        blk[0:64, 4 * tau + 1] = 1.0 / 64.0
        blk[64:128, 4 * tau + 2] = W[128]
        blk[64:128, 4 * tau + 3] = 1.0 / 64.0
    b1 = np.concatenate([nb[0:64], nb[0:64]]).reshape(128, 1).astype(np.float32)
    b2 = np.concatenate([nb[64:128], nb[64:128]]).reshape(128, 1).astype(np.float32)

    consts = {
        "w1": w1, "w2": w2, "wz": wz, "b1": b1, "b2": b2,
        "arA": np.ascontiguousarray(Ar[0:128]),
        "arB": np.ascontiguousarray(Ar[128:256]),
        "aiA": np.ascontiguousarray(Ai[0:128]),
        "aiB": np.ascontiguousarray(Ai[128:256]),
        "crm": np.ascontiguousarray(Cr[0:128]),
        "crn": np.ascontiguousarray(Cr[128:129]),
        "cim": np.ascontiguousarray(Ci[0:128]),
        "ident": np.eye(128, dtype=np.float32),
    }

    x = np.ascontiguousarray(x, np.float32)
    wn = np.ascontiguousarray(white_noise, np.float32)
    wn_pad = np.pad(wn, ((0, 0), (0, HOP)))
    in_maps = []
    for c in range(NCORES):
        xs = x[BLOC * c:BLOC * (c + 1)]                    # [4, 64, T]
        m = dict(consts)
        m["x"] = np.ascontiguousarray(xs.reshape(PAIRS, 128, T))
        m["wn"] = np.ascontiguousarray(wn_pad[BLOC * c:BLOC * (c + 1)])
        in_maps.append(m)
    return in_maps


def kernel(x, amp_w=None, amp_b=None, freq_w=None, freq_b=None,
           noise_w=None, noise_b=None, noise_factor=None, white_noise=None,
           **_unused):
    from concourse.bass_utils import run_bass_kernel_spmd

    key = "nc1"
    if key not in _CACHE:
        _CACHE[key] = _build(reps=1)
    nc = _CACHE[key]

    in_maps = _host_prep(np.asarray(x), np.asarray(noise_w),
                         np.asarray(noise_b), noise_factor,
                         np.asarray(white_noise))
    res = run_bass_kernel_spmd(nc, in_maps, core_ids=list(range(NCORES)))
    out = np.empty((B, 1, T), np.float32)
    for c in range(NCORES):
        out[BLOC * c:BLOC * (c + 1), 0, :] = res.results[c]["y"]
    return out


# revision 14
# speedup vs baseline: 1.3399x; 1.3399x over previous
"""Trainium2 (trn2) Bass kernel for the DDSP noise-synthesis module.

Problem (hardcoded; no external files read):
  x           [32, 64, 16384] f32
  noise_w     [129, 64], noise_b [129] (zeros), noise_factor scalar
  white_noise [32, 16384]
  out[b, 0, t] = mean_c x[b, c, t] + noise_factor * noise_bank(spec_b, white_b)[t]
  spec_b = avgpool_128(clip(noise_w @ x_b + noise_b, 0, 1))        # [129, 128]
  noise_bank: per-frame rFFT(256, ortho) filtering of white noise + 50% overlap-add.
  (The reference's amp/freq oscillator branch is computed-but-discarded dead code.)

Strategy: pure data parallel, 4 batches per core (2 pairs).  Per pair the
channel dims of two batches are stacked to 128 partitions and the 1x1 conv
runs as three stationary-weight matmul passes (coeffs 0-63, 64-127 both
batches block-diagonal; Nyquist+mean rows via 32-col masked stationaries
accumulated into one PSUM tile).  clip+pool: ScalarE relu(+bias)+bf16 cast,
VectorE min(1) + segmented reduce.  rFFT/irFFT are 128/256-point DFT matmuls
with the overlap-add folded into PSUM accumulation (shifted filter columns),
and noise_factor/pool/ortho scales folded into the iDFT constants.
"""

import numpy as np

B, CH, T = 32, 64, 16384
NCORES = 8
BLOC = B // NCORES          # 4 batches per core
PAIRS = BLOC // 2           # 2
K = 129                     # rfft coeffs
S = 128                     # frames
WIN = 256
HOP = 128
SEG = 128                   # pool window
XCH = 4096                  # x stream chunk (free elems)
NQ = T // XCH               # 4
CCH = 512                   # conv free chunk = 1 PSUM bank of f32
NT = T // CCH               # 32

_CACHE: dict = {}


def _dft_consts(noise_factor: float):
    n = np.arange(WIN)[:, None].astype(np.float64)
    k = np.arange(K)[None, :].astype(np.float64)
    ang = 2.0 * np.pi * n * k / WIN
    Ar = (np.cos(ang) / 16.0).astype(np.float32)           # [256, 129]
    Ai = (-np.sin(ang) / 16.0).astype(np.float32)
    wk = np.full((K,), 2.0)
    wk[0] = 1.0
    wk[K - 1] = 1.0
    scale = noise_factor / (16.0 * 128.0)                  # ortho irfft + pool mean
    ang2 = 2.0 * np.pi * np.arange(K)[:, None] * np.arange(WIN)[None, :] / WIN
    Cr = (wk[:, None] * np.cos(ang2) * scale).astype(np.float32)   # [129, 256]
    Ci = (-wk[:, None] * np.sin(ang2) * scale).astype(np.float32)
    return Ar, Ai, Cr, Ci


def _build(reps: int = 1):
    from contextlib import ExitStack

    import concourse.bacc as bacc
    import concourse.bass as bass
    import concourse.tile as tile
    from concourse import mybir

    f32 = mybir.dt.float32
    f32r = mybir.dt.float32r
    bf16 = mybir.dt.bfloat16
    AF = mybir.ActivationFunctionType
    ALU = mybir.AluOpType
    AX = mybir.AxisListType

    nc = bacc.Bacc("TRN2", target_bir_lowering=False, debug=False,
                   num_devices=NCORES)

    xd = nc.dram_tensor("x", [PAIRS, 128, T], f32, kind="ExternalInput")
    wnd = nc.dram_tensor("wn", [BLOC, T + HOP], f32, kind="ExternalInput")
    w1d = nc.dram_tensor("w1", [128, 128], f32, kind="ExternalInput")
    w2d = nc.dram_tensor("w2", [128, 128], f32, kind="ExternalInput")
    wzd = nc.dram_tensor("wz", [128, 256], f32, kind="ExternalInput")
    b1d = nc.dram_tensor("b1", [128, 1], f32, kind="ExternalInput")
    b2d = nc.dram_tensor("b2", [128, 1], f32, kind="ExternalInput")
    arAd = nc.dram_tensor("arA", [128, K], f32, kind="ExternalInput")
    arBd = nc.dram_tensor("arB", [128, K], f32, kind="ExternalInput")
    aiAd = nc.dram_tensor("aiA", [128, K], f32, kind="ExternalInput")
    aiBd = nc.dram_tensor("aiB", [128, K], f32, kind="ExternalInput")
    crmd = nc.dram_tensor("crm", [128, WIN], f32, kind="ExternalInput")
    crnd = nc.dram_tensor("crn", [1, WIN], f32, kind="ExternalInput")
    cimd = nc.dram_tensor("cim", [128, WIN], f32, kind="ExternalInput")
    identd = nc.dram_tensor("ident", [128, 128], f32, kind="ExternalInput")
    yd = nc.dram_tensor("y", [BLOC, T], f32, kind="ExternalOutput")

    with tile.TileContext(nc) as tc, ExitStack() as ctx:
        consts = ctx.enter_context(tc.tile_pool(name="consts", bufs=1))
        xpool = ctx.enter_context(tc.tile_pool(name="xp", bufs=2 * NQ))
        ring = ctx.enter_context(tc.tile_pool(name="ring", bufs=6))
        specp = ctx.enter_context(tc.tile_pool(name="spec", bufs=4))
        qp = ctx.enter_context(tc.tile_pool(name="qp", bufs=2))
        sb = ctx.enter_context(tc.tile_pool(name="sb", bufs=2))
        outp = ctx.enter_context(tc.tile_pool(name="outp", bufs=2))
        pmm = ctx.enter_context(tc.tile_pool(name="pmm", bufs=2, space="PSUM"))
        pp3 = ctx.enter_context(tc.tile_pool(name="pp3", bufs=1, space="PSUM"))
        pnz = ctx.enter_context(tc.tile_pool(name="pnz", bufs=3, space="PSUM"))

        def cload(dram, shape, tag, dt=f32):
            t = consts.tile(shape, dt, tag=tag)
            src = dram[:, :]
            if dt is f32r:
                src = src.bitcast(f32r)
            nc.sync.dma_start(out=t, in_=src)
            return t

        w1t = cload(w1d, [128, 128], "w1", f32r)
        w2t = cload(w2d, [128, 128], "w2", f32r)
        wzt = cload(wzd, [128, 256], "wz")
        b1t = cload(b1d, [128, 1], "b1")
        b2t = cload(b2d, [128, 1], "b2")
        arAt = cload(arAd, [128, K], "arA", f32r)
        arBt = cload(arBd, [128, K], "arB", f32r)
        aiAt = cload(aiAd, [128, K], "aiA", f32r)
        aiBt = cload(aiBd, [128, K], "aiB", f32r)
        crmt = cload(crmd, [128, WIN], "crm", f32r)
        crnt = cload(crnd, [1, WIN], "crn", f32r)
        cimt = cload(cimd, [128, WIN], "cim", f32r)
        identt = cload(identd, [128, 128], "ident")

        for _rep in range(reps):
            for pair in range(PAIRS):
                # ---- stream x for this batch pair ----
                xq = []
                for q in range(NQ):
                    t = xpool.tile([128, XCH], f32r)
                    nc.sync.dma_start(
                        out=t,
                        in_=xd[pair, :, q * XCH:(q + 1) * XCH].bitcast(f32r))
                    xq.append(t)

                # ---- conv passes 1/2: coeffs 0-63 / 64-127 (both batches) ----
                # float32r: single-pass fp32 matmul (vs LOW_HIGH 2-pass);
                # spec branch tolerates the reduced mantissa.
                spec = []
                for wt, bt in ((w1t, b1t), (w2t, b2t)):
                    sp = specp.tile([128, S], f32)
                    for ti in range(NT // 2):
                        q, off = divmod(ti * 2 * CCH, XCH)
                        ps = pmm.tile([128, 2 * CCH], f32)
                        nc.tensor.matmul(ps[:, 0:CCH], wt,
                                         xq[q][:, off:off + CCH],
                                         start=True, stop=True)
                        nc.tensor.matmul(ps[:, CCH:2 * CCH], wt,
                                         xq[q][:, off + CCH:off + 2 * CCH],
                                         start=True, stop=True)
                        rb = ring.tile([128, 2 * CCH], bf16)
                        nc.scalar.activation(rb, ps, AF.Relu, bias=bt, scale=1.0)
                        nc.vector.tensor_scalar_min(rb, rb, 1.0)
                        nc.vector.tensor_reduce(
                            sp[:, 8 * ti:8 * ti + 8],
                            rb.rearrange("p (a b) -> p a b", b=SEG),
                            axis=AX.X, op=ALU.add)
                    spec.append(sp)

                # ---- pass 3: [nyqA, meanA, nyqB, meanB] x 32 chunks ----
                p3 = pp3.tile([128, CCH], f32)
                for ti in range(NT):
                    g, tau = divmod(ti, 8)
                    q, off = divmod(ti * CCH, XCH)
                    nc.tensor.matmul(
                        p3[32 * g:32 * g + 32, :],
                        wzt[:, 32 * tau:32 * tau + 32],
                        xq[q][:, off:off + CCH].bitcast(f32),
                        start=(tau == 0), stop=(tau == 7),
                        tile_position=(0, 32 * g))
                qt = qp.tile([128, CCH], f32)
                nc.vector.tensor_copy(qt, p3)

                # ---- per-batch noise bank + output ----
                for i in range(2):
                    b = 2 * pair + i

                    # compact nyq/mean rows of this batch: [32 chunks, 512]
                    qn = sb.tile([32, CCH], f32, tag="qn")
                    nc.sync.dma_start(out=qn, in_=qt[2 * i::4, :])
                    qm = sb.tile([32, CCH], f32, tag="qm")
                    nc.sync.dma_start(out=qm, in_=qt[2 * i + 1::4, :])

                    # spec for this batch: [128 k, 128 s]
                    spb = sb.tile([128, S], f32, tag="spb")
                    nc.sync.dma_start(out=spb[0:64, :],
                                      in_=spec[0][64 * i:64 * i + 64, :])
                    nc.sync.dma_start(out=spb[64:128, :],
                                      in_=spec[1][64 * i:64 * i + 64, :])

                    # nyquist spec row: clip + pool on qn, then lay out [1, 128]
                    qnc = sb.tile([32, CCH], f32, tag="qnc")
                    nc.vector.tensor_scalar(qnc, qn, 0.0, 1.0,
                                            op0=ALU.max, op1=ALU.min)
                    sgrid = sb.tile([32, 4], f32, tag="sgrid")
                    nc.vector.tensor_reduce(
                        sgrid, qnc.rearrange("p (a b) -> p a b", b=SEG),
                        axis=AX.X, op=ALU.add)
                    nyqrow = sb.tile([1, S], f32, tag="nyqrow")
                    nc.sync.dma_start(
                        out=bass.AP(tensor=nyqrow.tensor, offset=nyqrow.offset,
                                    ap=[[1, 1], [4, 32], [1, 4]]),
                        in_=bass.AP(tensor=sgrid.tensor, offset=sgrid.offset,
                                    ap=[[1, 1]] + [list(d) for d in sgrid.ap]))

                    # frames of white noise (overlapped windows) + transpose
                    wn_b = wnd[b, :]
                    frames = sb.tile([S, WIN], f32, tag="frames")
                    nc.sync.dma_start(
                        out=frames,
                        in_=bass.AP(tensor=wn_b.tensor, offset=wn_b.offset,
                                    ap=[[HOP, S], [1, WIN]]))
                    fT = []
                    for h in range(2):
                        tr = pnz.tile([128, 128], f32, tag="ps_nz")
                        nc.tensor.transpose(
                            tr, frames[:, 128 * h:128 * h + 128], identt)
                        ft = sb.tile([128, 128], f32r, tag=f"ft{h}")
                        nc.vector.tensor_copy(ft, tr)
                        fT.append(ft)

                    # rfft: nf[k, s] (main 0..127 and nyquist row)
                    nfr = pnz.tile([128, 128], f32, tag="ps_nz")
                    nc.tensor.matmul(nfr, arAt[:, 0:128],
                                     fT[0], start=True, stop=False)
                    nc.tensor.matmul(nfr, arBt[:, 0:128],
                                     fT[1], start=False, stop=True)
                    nfi = pnz.tile([128, 128], f32, tag="ps_nz")
                    nc.tensor.matmul(nfi, aiAt[:, 0:128],
                                     fT[0], start=True, stop=False)
                    nc.tensor.matmul(nfi, aiBt[:, 0:128],
                                     fT[1], start=False, stop=True)
                    nfn = pnz.tile([1, 128], f32, tag="ps_nz")
                    nc.tensor.matmul(nfn, arAt[:, 128:129],
                                     fT[0], start=True, stop=False)
                    nc.tensor.matmul(nfn, arBt[:, 128:129],
                                     fT[1], start=False, stop=True)

                    # filt = nf * spec ; shifted copies for the OLA tail
                    fr = sb.tile([128, S], f32r, tag="fr")
                    nc.vector.tensor_mul(fr, nfr, spb)
                    fi = sb.tile([128, S], f32r, tag="fi")
                    nc.vector.tensor_mul(fi, nfi, spb)
                    fn = sb.tile([1, S], f32r, tag="fn")
                    nc.vector.tensor_mul(fn, nfn, nyqrow)

                    # transposed irfft + OLA (noise_factor prescaled in C*):
                    # head[j, s] and tail[j, s] accumulate separately; the
                    # 50% overlap-add is tail[s-1] added during the combine.
                    olaT = pnz.tile([128, 128], f32, tag="ps_nz")
                    nc.tensor.matmul(olaT, crmt[:, 0:128],
                                     fr, start=True, stop=False)
                    nc.tensor.matmul(olaT, cimt[:, 0:128],
                                     fi, start=False, stop=False)
                    nc.tensor.matmul(olaT, crnt[:, 0:128],
                                     fn, start=False, stop=True)
                    tlT = pnz.tile([128, 128], f32, tag="ps_nz")
                    nc.tensor.matmul(tlT, crmt[:, 128:256],
                                     fr, start=True, stop=False)
                    nc.tensor.matmul(tlT, cimt[:, 128:256],
                                     fi, start=False, stop=False)
                    nc.tensor.matmul(tlT, crnt[:, 128:256],
                                     fn, start=False, stop=True)
                    tl = sb.tile([128, 128], f32, tag="tl")
                    nc.vector.tensor_copy(tl, tlT)

                    # mean rows -> mean_T[j, (u, t)] via PE transposes
                    meanT = pnz.tile([128, 128], f32, tag="ps_nz")
                    for u in range(4):
                        nc.tensor.transpose(
                            meanT[:, 32 * u:32 * u + 32],
                            qm[:, 128 * u:128 * u + 128],
                            identt[0:32, 0:32])
                    meanTs = sb.tile([128, 128], f32, tag="meanTs")
                    nc.vector.tensor_copy(meanTs, meanT)

                    # F[j, s] = olaT + meanT  (s = 4t + u; meanT free is (u, t))
                    F = sb.tile([128, 128], f32, tag="F")
                    nc.vector.tensor_add(
                        bass.AP(tensor=F.tensor, offset=F.offset,
                                ap=[list(F.ap[0]), [4, 32], [1, 4]]),
                        bass.AP(tensor=olaT.tensor, offset=olaT.offset,
                                ap=[list(olaT.ap[0]), [4, 32], [1, 4]]),
                        bass.AP(tensor=meanTs.tensor, offset=meanTs.offset,
                                ap=[list(meanTs.ap[0]), [1, 32], [32, 4]]))
                    # overlap-add tail: F[:, 1:] += tail[:, :-1]
                    nc.vector.tensor_add(F[:, 1:S], F[:, 1:S], tl[:, 0:S - 1])

                    # transpose back to [s, j] and store
                    Ft = pnz.tile([128, 128], f32, tag="ps_nz")
                    nc.tensor.transpose(Ft, F, identt)
                    osb = outp.tile([128, 128], f32)
                    nc.vector.tensor_copy(osb, Ft)
                    yb = yd[b, :]
                    nc.sync.dma_start(
                        out=bass.AP(tensor=yb.tensor, offset=yb.offset,
                                    ap=[[128, 128], [1, 128]]),
                        in_=osb)

    nc.compile()
    return nc


def _host_prep(x, noise_w, noise_b, noise_factor, white_noise):
    W = np.ascontiguousarray(noise_w, np.float32)          # [129, 64]
    nb = np.asarray(noise_b, np.float32)
    nf = float(np.asarray(noise_factor, np.float32))
    Ar, Ai, Cr, Ci = _dft_consts(nf)

    w1 = np.zeros((128, 128), np.float32)
    w1[0:64, 0:64] = W[0:64].T
    w1[64:128, 64:128] = W[0:64].T
    w2 = np.zeros((128, 128), np.float32)
    w2[0:64, 0:64] = W[64:128].T
    w2[64:128, 64:128] = W[64:128].T
    wz = np.zeros((128, 256), np.float32)
    for tau in range(8):
        blk = wz[:, 32 * tau:32 * tau + 32]
        blk[0:64, 4 * tau + 0] = W[128]
        blk[0:64, 4 * tau + 1] = 1.0 / 64.0
        blk[64:128, 4 * tau + 2] = W[128]
        blk[64:128, 4 * tau + 3] = 1.0 / 64.0
    b1 = np.concatenate([nb[0:64], nb[0:64]]).reshape(128, 1).astype(np.float32)
    b2 = np.concatenate([nb[64:128], nb[64:128]]).reshape(128, 1).astype(np.float32)

    consts = {
        "w1": w1, "w2": w2, "wz": wz, "b1": b1, "b2": b2,
        "arA": np.ascontiguousarray(Ar[0:128]),
        "arB": np.ascontiguousarray(Ar[128:256]),
        "aiA": np.ascontiguousarray(Ai[0:128]),
        "aiB": np.ascontiguousarray(Ai[128:256]),
        "crm": np.ascontiguousarray(Cr[0:128]),
        "crn": np.ascontiguousarray(Cr[128:129]),
        "cim": np.ascontiguousarray(Ci[0:128]),
        "ident": np.eye(128, dtype=np.float32),
    }

    x = np.ascontiguousarray(x, np.float32)
    wn = np.ascontiguousarray(white_noise, np.float32)
    wn_pad = np.pad(wn, ((0, 0), (0, HOP)))
    in_maps = []
    for c in range(NCORES):
        xs = x[BLOC * c:BLOC * (c + 1)]                    # [4, 64, T]
        m = dict(consts)
        m["x"] = np.ascontiguousarray(xs.reshape(PAIRS, 128, T))
        m["wn"] = np.ascontiguousarray(wn_pad[BLOC * c:BLOC * (c + 1)])
        in_maps.append(m)
    return in_maps


def kernel(x, amp_w=None, amp_b=None, freq_w=None, freq_b=None,
           noise_w=None, noise_b=None, noise_factor=None, white_noise=None,
           **_unused):
    from concourse.bass_utils import run_bass_kernel_spmd

    key = "nc1"
    if key not in _CACHE:
        _CACHE[key] = _build(reps=1)
    nc = _CACHE[key]

    in_maps = _host_prep(np.asarray(x), np.asarray(noise_w),
                         np.asarray(noise_b), noise_factor,
                         np.asarray(white_noise))
    res = run_bass_kernel_spmd(nc, in_maps, core_ids=list(range(NCORES)))
    out = np.empty((B, 1, T), np.float32)
    for c in range(NCORES):
        out[BLOC * c:BLOC * (c + 1), 0, :] = res.results[c]["y"]
    return out
